# revision 3
# baseline (speedup 1.0000x reference)
"""Causal multi-head attention (B=4, T=2048, D=512, H=8) on 8 TRN2 NeuronCores.

Sharding: core c handles batch b = c//2 and heads [4*(c%2), 4*(c%2)+4).
Data parallel on B (4 batches x 2 cores each), tensor parallel on H
(w_qkv column-sharded, w_proj row-sharded). Each core produces a partial
projection output yT [512, 2048] (f32); the host sums the two partials per
batch, transposes, and adds b_proj.

On-device layout (per core, all compute in fp16 with f32 PSUM accumulation):
  xT  [512, 2048]  x[b]^T                 (host pre-transposed, fp16)
  wq  [512, 256]   (Wq_shard * 1/sqrt(K))^T   (lhsT for qkvT matmul)
  wk  [512, 256]   Wk_shard^T
  wv  [512, 256]   Wv_shard^T             (rhs for v matmul)
  wp  [256, 512]   Wp_shard^T             (lhsT for proj)

  qT/kT (SBUF): [128, 2, 2048] fp16 - head h lives at partitions
     64*(h%2):64*(h%2)+64 of tile h//2, so QK matmuls for odd heads run at
     tile_position (64, 0) (row-packed PE usage).
  v (SBUF): [128, 16, 260] fp16 - per t-block, per head: 64 v columns + a
     ones column (cols 65h..65h+65) so AV's lhsT [v|1] emits the softmax
     denominator as output partition 64.

  Attention (per head, flash-style over s-blocks of 128):
    scoresT[s, q] = k_blk @ qT  -> PSUM f32 (causal windows, 1024-col halves)
    expT = exp(scoresT)         -> SBUF fp16 (ACT), triangular-masked on diag
    outT[65, q] += [v|1]^T-style matmul accumulating over s-blocks
    zn = outT[0:64] * recip(outT[64]) broadcast  -> proj rhs fp16
  proj: yT[512, 2048] = wp^T.T @ zn -> DMA out (f32).
"""

import numpy as np

import concourse.bass as bass
import concourse.mybir as mybir
from concourse.bacc import Bacc
from concourse.tile import TileContext

F16 = mybir.dt.float16
F32 = mybir.dt.float32
FT = mybir.ActivationFunctionType
OP = mybir.AluOpType

T = 2048
D = 512
HPC = 4  # heads per core
K = 64  # head dim
P = 128
NSB = T // P  # 16 s-blocks


def build_nc():
    nc = Bacc()

    xT = nc.declare_dram_parameter("xT", [D, T], F16, isOutput=False)
    wq = nc.declare_dram_parameter("wq", [D, 256], F16, isOutput=False)
    wk = nc.declare_dram_parameter("wk", [D, 256], F16, isOutput=False)
    wv = nc.declare_dram_parameter("wv", [D, 256], F16, isOutput=False)
    wp = nc.declare_dram_parameter("wp", [256, D], F16, isOutput=False)
    yT = nc.declare_dram_parameter("yT", [D, T], F32, isOutput=True)

    with TileContext(nc) as tc:
        with (
            tc.tile_pool(name="persist", bufs=1) as pp,
            tc.tile_pool(name="work", bufs=3) as wk_pool,
            tc.tile_pool(name="ps", bufs=2, space="PSUM") as ps_pool,
            tc.tile_pool(name="outT_ps", bufs=1, space="PSUM") as outT_pool,
        ):
            # ---- persistent SBUF tensors ----
            xT_sb = pp.tile([P, 4, T], F16, tag="xT_sb")
            qT_sb = pp.tile([P, 2, T], F16, tag="qT_sb")
            kT_sb = pp.tile([P, 2, T], F16, tag="kT_sb")
            v_sb = pp.tile([P, NSB, HPC * (K + 1)], F16, tag="v_sb")
            zn_sb = pp.tile([P, 2, T], F16, tag="zn_sb")
            yT_sb = pp.tile([P, 4, T], F32, tag="yT_sb")
            wq_sb = pp.tile([P, 4, 256], F16, tag="wq_sb")
            wk_sb = pp.tile([P, 4, 256], F16, tag="wk_sb")
            wv_sb = pp.tile([P, 4, 256], F16, tag="wv_sb")
            wp_sb = pp.tile([P, 2, D], F16, tag="wp_sb")
            trimask = pp.tile([P, P], F16, tag="trimask")

            # ---- input DMAs ----
            nc.sync.dma_start(out=xT_sb[:], in_=xT.rearrange("(c p) t -> p c t", p=P))
            nc.sync.dma_start(out=wq_sb[:], in_=wq.rearrange("(c p) n -> p c n", p=P))
            nc.sync.dma_start(out=wk_sb[:], in_=wk.rearrange("(c p) n -> p c n", p=P))
            nc.sync.dma_start(out=wv_sb[:], in_=wv.rearrange("(c p) n -> p c n", p=P))
            nc.sync.dma_start(out=wp_sb[:], in_=wp.rearrange("(c p) n -> p c n", p=P))

            # ---- constants ----
            # trimask[p, f] = 1 if f >= p else 0 (keep q >= s on the diagonal block)
            nc.gpsimd.memset(trimask[:], 1.0)
            nc.gpsimd.affine_select(
                out=trimask[:],
                in_=trimask[:],
                compare_op=OP.is_ge,
                fill=0.0,
                base=0,
                pattern=[[1, P]],
                channel_multiplier=-1,
            )
            # ones columns of v (col 65h+64 per head)
            for h in range(HPC):
                nc.gpsimd.memset(v_sb[:, :, h * (K + 1) + K : h * (K + 1) + K + 1], 1.0)

            # ---- qkv projections ----
            # qT/kT: out[e, t] for e-tile pt, 512-col chunk ch, contraction c
            for w_sb, dest, eng in ((wq_sb, qT_sb, "v"), (wk_sb, kT_sb, "s")):
                for pt in range(2):
                    for ch in range(4):
                        ps = ps_pool.tile([P, 512], F32, tag="ps")
                        for c in range(4):
                            nc.tensor.matmul(
                                ps[:],
                                lhsT=w_sb[:, c, 128 * pt : 128 * (pt + 1)],
                                rhs=xT_sb[:, c, 512 * ch : 512 * (ch + 1)],
                                start=(c == 0),
                                stop=(c == 3),
                            )
                        dst = dest[:, pt, 512 * ch : 512 * (ch + 1)]
                        if eng == "v":
                            nc.vector.tensor_copy(dst, ps[:])
                        else:
                            nc.scalar.copy(dst, ps[:])
            # v: out[t-block, e] with per-head interleave (65-stride)
            for tb in range(NSB):
                ps = ps_pool.tile([P, 256], F32, tag="ps")
                for c in range(4):
                    nc.tensor.matmul(
                        ps[:],
                        lhsT=xT_sb[:, c, 128 * tb : 128 * (tb + 1)],
                        rhs=wv_sb[:, c, :],
                        start=(c == 0),
                        stop=(c == 3),
                    )
                nc.vector.tensor_copy(
                    v_sb[:, tb, :].rearrange("p (h c) -> p h c", c=K + 1)[:, :, 0:K],
                    ps.rearrange("p (h c) -> p h c", c=K),
                )

            # ---- attention, one head at a time ----
            for h in range(HPC):
                po = 64 * (h % 2)
                tpl = h // 2
                outT = outT_pool.tile([K + 1, T], F32, tag="outT")
                for i in range(NSB):
                    wlo = P * i  # causal window = [wlo, T)
                    ex = wk_pool.tile([P, T], F16, tag="expT")
                    for j in (0, 1):  # 1024-col score halves
                        jlo, jhi = 1024 * j, 1024 * (j + 1)
                        lo = max(wlo, jlo)
                        if lo >= jhi:
                            continue
                        sc = ps_pool.tile([P, 1024], F32, tag="ps")
                        p0 = lo
                        while p0 < jhi:  # QK pieces, 512-grid (PSUM bank limit)
                            p1 = min(jhi, (p0 // 512 + 1) * 512)
                            nc.tensor.matmul(
                                sc[:, p0 - jlo : p1 - jlo],
                                lhsT=kT_sb[po : po + 64, tpl, wlo : wlo + P],
                                rhs=qT_sb[po : po + 64, tpl, p0:p1],
                                start=True,
                                stop=True,
                            )
                            p0 = p1
                        nc.scalar.activation(
                            out=ex[:, lo:jhi], in_=sc[:, lo - jlo : jhi - jlo], func=FT.Exp
                        )
                    # causal mask on the diagonal block
                    nc.vector.tensor_tensor(
                        ex[:, wlo : wlo + P], ex[:, wlo : wlo + P], trimask[:], OP.mult
                    )
                    # AV accumulation (with denominator via the ones column)
                    p0 = wlo
                    while p0 < T:
                        jb = p0 // 512
                        p1 = min(T, 512 * (jb + 1))
                        nc.tensor.matmul(
                            outT[:, p0:p1],
                            lhsT=v_sb[:, i, h * (K + 1) : (h + 1) * (K + 1)],
                            rhs=ex[:, p0:p1],
                            start=(i == 0),
                            stop=(i == min(NSB - 1, 4 * jb + 3)),
                        )
                        p0 = p1
                # normalize: zn = outT[0:64] / outT[64]
                den = wk_pool.tile([1, T], F32, tag="den")
                nc.vector.tensor_copy(den[:], outT[K : K + 1, :])
                rec = wk_pool.tile([1, T], F32, tag="rec")
                nc.vector.reciprocal_approx_fast(out=rec[0:1, :], in_=den[0:1, :])
                recb = wk_pool.tile([64, T], F32, tag="recb")
                nc.gpsimd.partition_broadcast(recb[:], rec[0:1, :])
                nc.vector.tensor_tensor(
                    zn_sb[po : po + 64, tpl, :], outT[0:K, :], recb[:], OP.mult
                )

            # ---- output projection: yT = wp^T.T @ zn ----
            for m in range(4):
                for w in range(4):
                    ps = ps_pool.tile([P, 512], F32, tag="ps")
                    for c in range(2):
                        nc.tensor.matmul(
                            ps[:],
                            lhsT=wp_sb[:, c, 128 * m : 128 * (m + 1)],
                            rhs=zn_sb[:, c, 512 * w : 512 * (w + 1)],
                            start=(c == 0),
                            stop=(c == 1),
                        )
                    nc.vector.tensor_copy(yT_sb[:, m, 512 * w : 512 * (w + 1)], ps[:])

            nc.sync.dma_start(out=yT.rearrange("(m p) t -> p m t", p=P), in_=yT_sb[:])

    nc.finalize()
    return nc


_NC = None


def _get_nc():
    global _NC
    if _NC is None:
        _NC = build_nc()
    return _NC


def make_in_maps(x, w_qkv, w_proj):
    x = np.asarray(x, dtype=np.float32)
    w_qkv = np.asarray(w_qkv, dtype=np.float32)
    w_proj = np.asarray(w_proj, dtype=np.float32)
    in_maps = []
    for c in range(8):
        b = c // 2
        h0 = 4 * (c % 2)
        r = slice(64 * h0, 64 * h0 + 256)
        scale = float(K) ** -0.5
        wq = np.ascontiguousarray((w_qkv[0:512][r] * scale).T).astype(np.float16)
        wkm = np.ascontiguousarray(w_qkv[512:1024][r].T).astype(np.float16)
        wvm = np.ascontiguousarray(w_qkv[1024:1536][r].T).astype(np.float16)
        wpm = np.ascontiguousarray(w_proj[:, r].T).astype(np.float16)
        xT = np.ascontiguousarray(x[b].T).astype(np.float16)
        in_maps.append({"xT": xT, "wq": wq, "wk": wkm, "wv": wvm, "wp": wpm})
    return in_maps


def assemble_output(results, b_proj):
    b_proj = np.asarray(b_proj, dtype=np.float32)
    y = np.empty((4, T, D), np.float32)
    for b in range(4):
        yTc = results[2 * b]["yT"].astype(np.float32) + results[2 * b + 1]["yT"].astype(
            np.float32
        )
        y[b] = yTc.T + b_proj
    return y


def kernel(x, w_qkv, w_proj, b_proj):
    from concourse.bass_utils import run_bass_kernel_spmd

    nc = _get_nc()
    in_maps = make_in_maps(x, w_qkv, w_proj)
    res = run_bass_kernel_spmd(nc, in_maps, core_ids=list(range(8)))
    return assemble_output(res.results, b_proj)


# revision 6
# speedup vs baseline: 1.2634x; 1.2634x over previous
"""Causal multi-head attention (B=4, T=2048, D=512, H=8) on 8 TRN2 NeuronCores.

Sharding: core c handles batch b = c//2 and heads [4*(c%2), 4*(c%2)+4).
Data parallel on B (4 batches x 2 cores each), tensor parallel on H
(w_qkv column-sharded, w_proj row-sharded). Each core produces a partial
projection output yT [512, 2048] (f32); the host sums the two partials per
batch, transposes, and adds b_proj.

On-device layout (per core, fp16 compute, f32 PSUM accumulation):
  qT/kT (SBUF): [128, 2, 2048] fp16 - head h at partitions 64*(h%2)..+64 of
     tile h//2. QK matmuls for an even/odd head pair run in the two 64-row
     halves of the PE array concurrently (tile_position row packing).
  v (SBUF): [128, 16, 260] fp16 - per t-block, per head: 64 v columns + a
     ones column, so AV's lhsT [v|1] emits the softmax denominator as
     output partition 64.

  Attention runs per head-pair (0,1) then (2,3), per q-half (cols 0:1024,
  1024:2048) so that two f32 outT accumulators [65, 1024] plus two score
  tiles [128, 1024] fit the 8 PSUM banks:
    scoresT[s, q] = k_blk @ qT  (causal windows)  -> exp on ACT -> expT fp16
    diagonal 128x128 block triangular-masked (DVE multiply)
    outT[65, q] += [v|1]-matmul over s-blocks; normalize by recip(denom).
  proj: yT[512, 2048] = wp^T.T @ zn, 1MB output DMA per column chunk.
"""

import numpy as np

import concourse.bass as bass
import concourse.mybir as mybir
from concourse.bacc import Bacc
from concourse.tile import TileContext

F16 = mybir.dt.float16
F32 = mybir.dt.float32
FT = mybir.ActivationFunctionType
OP = mybir.AluOpType

T = 2048
D = 512
HPC = 4  # heads per core
K = 64  # head dim
P = 128
NSB = T // P  # 16 s-blocks
HALF = 1024


def build_nc():
    nc = Bacc()

    xT = nc.declare_dram_parameter("xT", [D, T], F16, isOutput=False)
    wq = nc.declare_dram_parameter("wq", [D, 256], F16, isOutput=False)
    wk = nc.declare_dram_parameter("wk", [D, 256], F16, isOutput=False)
    wv = nc.declare_dram_parameter("wv", [D, 256], F16, isOutput=False)
    wp = nc.declare_dram_parameter("wp", [256, D], F16, isOutput=False)
    yT = nc.declare_dram_parameter("yT", [D, T], F32, isOutput=True)

    with TileContext(nc) as tc:
        with (
            tc.tile_pool(name="persist", bufs=1) as pp,
            tc.tile_pool(name="work", bufs=4) as wkp,
            tc.tile_pool(name="ps", bufs=2, space="PSUM") as ps_pool,
            tc.tile_pool(name="outT_ps", bufs=2, space="PSUM") as outT_pool,
        ):
            # ---- persistent SBUF tensors ----
            xT_sb = pp.tile([P, 4, T], F16, tag="xT_sb")
            qT_sb = pp.tile([P, 2, T], F16, tag="qT_sb")
            kT_sb = pp.tile([P, 2, T], F16, tag="kT_sb")
            v_sb = pp.tile([P, NSB, HPC * (K + 1)], F16, tag="v_sb")
            zn_sb = pp.tile([P, 2, T], F16, tag="zn_sb")
            yT_sb = pp.tile([P, 4, T], F32, tag="yT_sb")
            wq_sb = pp.tile([P, 4, 256], F16, tag="wq_sb")
            wk_sb = pp.tile([P, 4, 256], F16, tag="wk_sb")
            wv_sb = pp.tile([P, 4, 256], F16, tag="wv_sb")
            wp_sb = pp.tile([P, 2, D], F16, tag="wp_sb")
            trimask = pp.tile([P, P], F16, tag="trimask")
            warm_sb = pp.tile([P, P], F16, tag="warm_sb")

            xT_dram = xT.rearrange("(c p) t -> p c t", p=P)

            # ---- PE warm-up fodder (keeps HAM busy during input DMA) ----
            # shares the outT tag's PSUM slots (released before attention)
            nc.gpsimd.memset(warm_sb[:], 0.0)
            warm_ps = outT_pool.tile([P, P], F32, tag="outT")

            # ---- input DMAs (weights first, xT in 512-col chunks) ----
            nc.sync.dma_start(out=wq_sb[:], in_=wq.rearrange("(c p) n -> p c n", p=P))
            nc.scalar.dma_start(out=wk_sb[:], in_=wk.rearrange("(c p) n -> p c n", p=P))
            nc.scalar.dma_start(out=wv_sb[:], in_=wv.rearrange("(c p) n -> p c n", p=P))
            nc.scalar.dma_start(out=wp_sb[:], in_=wp.rearrange("(c p) n -> p c n", p=P))
            for w in range(4):
                nc.sync.dma_start(
                    out=xT_sb[:, :, 512 * w : 512 * (w + 1)],
                    in_=xT_dram[:, :, 512 * w : 512 * (w + 1)],
                )

            for _ in range(40):
                nc.tensor.matmul(
                    warm_ps[:], lhsT=warm_sb[:], rhs=warm_sb[:], start=True, stop=True
                )

            # ---- constants ----
            # trimask[p, f] = 1 if f >= p else 0 (keep q >= s on the diagonal block)
            nc.gpsimd.memset(trimask[:], 1.0)
            nc.gpsimd.affine_select(
                out=trimask[:],
                in_=trimask[:],
                compare_op=OP.is_ge,
                fill=0.0,
                base=0,
                pattern=[[1, P]],
                channel_multiplier=-1,
            )
            for h in range(HPC):
                nc.gpsimd.memset(v_sb[:, :, h * (K + 1) + K : h * (K + 1) + K + 1], 1.0)

            # ---- qkv projections (all PSUM evacs on DVE; ACT is for exp) ----
            for w_sb, dest in ((wq_sb, qT_sb), (wk_sb, kT_sb)):
                for ch in range(4):
                    for pt in range(2):
                        ps = ps_pool.tile([P, 512], F32, tag="ps")
                        for c in range(4):
                            nc.tensor.matmul(
                                ps[:],
                                lhsT=w_sb[:, c, 128 * pt : 128 * (pt + 1)],
                                rhs=xT_sb[:, c, 512 * ch : 512 * (ch + 1)],
                                start=(c == 0),
                                stop=(c == 3),
                            )
                        nc.vector.tensor_copy(
                            dest[:, pt, 512 * ch : 512 * (ch + 1)], ps[:]
                        )
            for tb in range(NSB):
                ps = ps_pool.tile([P, 256], F32, tag="ps")
                for c in range(4):
                    nc.tensor.matmul(
                        ps[:],
                        lhsT=xT_sb[:, c, 128 * tb : 128 * (tb + 1)],
                        rhs=wv_sb[:, c, :],
                        start=(c == 0),
                        stop=(c == 3),
                    )
                nc.vector.tensor_copy(
                    v_sb[:, tb, :].rearrange("p (h c) -> p h c", c=K + 1)[:, :, 0:K],
                    ps.rearrange("p (h c) -> p h c", c=K),
                )

            # ---- attention: head pairs x q-halves ----
            for hp in range(2):  # head pair (2hp, 2hp+1)
                for qh in range(2):  # q-half: cols [1024*qh, 1024*(qh+1))
                    qlo, qhi = HALF * qh, HALF * (qh + 1)
                    n_i = 8 * qh + 8  # s-blocks 0..n_i-1 reach this q-half
                    outs = [
                        outT_pool.tile(
                            [K + 1, HALF], F32, tag="outT", name=f"outT_{hp}_{qh}_{hh}"
                        )
                        for hh in range(2)
                    ]
                    for i in range(n_i):
                        wlo = max(P * i, qlo)  # window in this half
                        exs = []
                        for hh in range(2):  # heads 2hp (po=0), 2hp+1 (po=64)
                            h = 2 * hp + hh
                            po = 64 * hh
                            sc = ps_pool.tile([P, HALF], F32, tag="ps")
                            p0 = wlo
                            while p0 < qhi:  # QK pieces, 512-grid
                                p1 = min(qhi, (p0 // 512 + 1) * 512)
                                nc.tensor.matmul(
                                    sc[:, p0 - qlo : p1 - qlo],
                                    lhsT=kT_sb[po : po + 64, hp, P * i : P * (i + 1)],
                                    rhs=qT_sb[po : po + 64, hp, p0:p1],
                                    start=True,
                                    stop=True,
                                )
                                p0 = p1
                            ex = wkp.tile([P, HALF], F16, tag="expT")
                            nc.scalar.activation(
                                out=ex[:, wlo - qlo :],
                                in_=sc[:, wlo - qlo :],
                                func=FT.Exp,
                            )
                            exs.append(ex)
                        for hh in range(2):
                            h = 2 * hp + hh
                            ex = exs[hh]
                            if P * i >= qlo:  # diagonal block lives in this half
                                nc.vector.tensor_tensor(
                                    ex[:, wlo - qlo : wlo - qlo + P],
                                    ex[:, wlo - qlo : wlo - qlo + P],
                                    trimask[:],
                                    OP.mult,
                                )
                            p0 = wlo
                            while p0 < qhi:  # AV pieces, 512-grid
                                jb = (p0 - qlo) // 512
                                p1 = min(qhi, qlo + 512 * (jb + 1))
                                bank_last = min(n_i - 1, (qlo + 512 * jb) // P + 3)
                                nc.tensor.matmul(
                                    outs[hh][:, p0 - qlo : p1 - qlo],
                                    lhsT=v_sb[:, i, h * (K + 1) : (h + 1) * (K + 1)],
                                    rhs=ex[:, p0 - qlo : p1 - qlo],
                                    start=(i == 0),
                                    stop=(i == bank_last),
                                )
                                p0 = p1
                    # normalize: zn = outT[0:64] / outT[64]
                    for hh in range(2):
                        h = 2 * hp + hh
                        po = 64 * hh
                        den = wkp.tile([1, HALF], F32, tag="den")
                        nc.vector.tensor_copy(den[:], outs[hh][K : K + 1, :])
                        rec = wkp.tile([1, HALF], F32, tag="rec")
                        nc.vector.reciprocal_approx_fast(
                            out=rec[0:1, :], in_=den[0:1, :]
                        )
                        recb = wkp.tile([64, HALF], F32, tag="recb")
                        nc.gpsimd.partition_broadcast(recb[:], rec[0:1, :])
                        nc.vector.tensor_tensor(
                            zn_sb[po : po + 64, hp, qlo:qhi],
                            outs[hh][0:K, :],
                            recb[:],
                            OP.mult,
                        )

            # ---- output projection + chunked output DMA ----
            for w in range(4):
                for m in range(4):
                    ps = ps_pool.tile([P, 512], F32, tag="ps")
                    for c in range(2):
                        nc.tensor.matmul(
                            ps[:],
                            lhsT=wp_sb[:, c, 128 * m : 128 * (m + 1)],
                            rhs=zn_sb[:, c, 512 * w : 512 * (w + 1)],
                            start=(c == 0),
                            stop=(c == 1),
                        )
                    if m % 2 == 0:
                        nc.vector.tensor_copy(yT_sb[:, m, 512 * w : 512 * (w + 1)], ps[:])
                    else:
                        nc.scalar.copy(yT_sb[:, m, 512 * w : 512 * (w + 1)], ps[:])
                nc.sync.dma_start(
                    out=yT.rearrange("(m p) t -> p m t", p=P)[:, :, 512 * w : 512 * (w + 1)],
                    in_=yT_sb[:, :, 512 * w : 512 * (w + 1)],
                )

    nc.finalize()
    return nc


_NC = None


def _get_nc():
    global _NC
    if _NC is None:
        _NC = build_nc()
    return _NC


def make_in_maps(x, w_qkv, w_proj):
    x = np.asarray(x, dtype=np.float32)
    w_qkv = np.asarray(w_qkv, dtype=np.float32)
    w_proj = np.asarray(w_proj, dtype=np.float32)
    in_maps = []
    for c in range(8):
        b = c // 2
        h0 = 4 * (c % 2)
        r = slice(64 * h0, 64 * h0 + 256)
        scale = float(K) ** -0.5
        wqm = np.ascontiguousarray((w_qkv[0:512][r] * scale).T).astype(np.float16)
        wkm = np.ascontiguousarray(w_qkv[512:1024][r].T).astype(np.float16)
        wvm = np.ascontiguousarray(w_qkv[1024:1536][r].T).astype(np.float16)
        wpm = np.ascontiguousarray(w_proj[:, r].T).astype(np.float16)
        xTm = np.ascontiguousarray(x[b].T).astype(np.float16)
        in_maps.append({"xT": xTm, "wq": wqm, "wk": wkm, "wv": wvm, "wp": wpm})
    return in_maps


def assemble_output(results, b_proj):
    b_proj = np.asarray(b_proj, dtype=np.float32)
    y = np.empty((4, T, D), np.float32)
    for b in range(4):
        yTc = results[2 * b]["yT"].astype(np.float32) + results[2 * b + 1]["yT"].astype(
            np.float32
        )
        y[b] = yTc.T + b_proj
    return y


def kernel(x, w_qkv, w_proj, b_proj):
    from concourse.bass_utils import run_bass_kernel_spmd

    nc = _get_nc()
    in_maps = make_in_maps(x, w_qkv, w_proj)
    res = run_bass_kernel_spmd(nc, in_maps, core_ids=list(range(8)))
    return assemble_output(res.results, b_proj)
